# revision 1
# baseline (speedup 1.0000x reference)
"""Masked multi-variant GRU (nn_MiddleLayer_30545807409998) on 8 TRN2 cores.

Strategy (self-contained; shapes hardcoded):
- Tensor-parallel over the 3*units gate dimension: core k owns gate columns
  [k*256,(k+1)*256) of each of z/r/h (768 U/W columns, host pre-sliced) and
  the matching h slice (rows [k*256,(k+1)*256) of h^T).
- Per time step: AllGather of the bf16 h^T slices across the 8 cores, then
  rec^T = (U_k)^T @ h^T on the TensorEngine (stationary = U tiles, full
  128x128), gate math on ACT/DVE in feature-major layout
  [feature partitions, vb free], vb = variant*8 + batch (64 columns).
- Input projection uses the mask-cumsum identity: the M=8 lower-triangular
  mask variants satisfy xp_v = sum_{j<=v} x_j @ W_j (j = maxlen block), an
  8x FLOP saving; partial sums accumulate in PSUM and each prefix is
  snapshotted to SBUF.
- Keras reset_after GRU bias handling: z/r gates get (b_in+b_rec) inside the
  sigmoid; the h gate adds b_rec BEFORE the r* multiply and b_in inside tanh.
- Matmuls in bf16 (fp32 PSUM accumulation), gate math and h state in fp32;
  measured absmax relative error ~7e-3 vs the fp32 reference.

Host side (this file): slices weight columns per core, transposes x/h0 to
the device layouts (pure gathers), launches the SPMD program via
run_bass_kernel_spmd, and reassembles ret/state from per-core outputs.
"""
import numpy as np

NCORES = 8
B, T, M, D = 8, 32, 8, 256
UNITS = M * D              # 2048
GPC = 3 * UNITS // NCORES  # 768 gate cols per core
VB = M * B                 # 64 folded (variant, batch)
KT = UNITS // 128          # 16 k-tiles

_CACHE = {}


def _build():
    import concourse.bacc as bacc
    import concourse.tile as tile
    import concourse.mybir as mybir
    F32 = mybir.dt.float32
    BF16 = mybir.dt.bfloat16
    AF = mybir.ActivationFunctionType

    nc = bacc.Bacc("TRN2", target_bir_lowering=False, debug=False,
                   num_devices=NCORES)
    xt_d = nc.dram_tensor("xt", [128, KT, 256], F32, kind="ExternalInput").ap()
    h0t_d = nc.dram_tensor("h0t", [128, KT, B], F32,
                           kind="ExternalInput").ap()
    h0k_d = nc.dram_tensor("h0k", [128, 2, B], F32, kind="ExternalInput").ap()
    w_d = nc.dram_tensor("wk", [UNITS, GPC], F32, kind="ExternalInput").ap()
    u_d = nc.dram_tensor("uk", [UNITS, GPC], F32, kind="ExternalInput").ap()
    b_d = nc.dram_tensor("bk", [128, 12], F32, kind="ExternalInput").ap()
    hs_d = nc.dram_tensor("hs", [T, 2, 128, VB], F32,
                          kind="ExternalOutput").ap()

    with tile.TileContext(nc) as tc:
        with tc.tile_pool(name="persist", bufs=1) as pp, \
             tc.tile_pool(name="stage", bufs=2) as sp, \
             tc.tile_pool(name="work", bufs=3) as wp, \
             tc.tile_pool(name="dram", bufs=1, space="DRAM") as dp:

            u_sb = pp.tile([128, KT * GPC], BF16)
            w_sb = pp.tile([128, KT * GPC], BF16)
            xt_sb = pp.tile([128, KT * 256], BF16)   # [(j,kd)*256 + b*32+t]
            xp_sb = pp.tile([128, T * 6 * VB], F32)  # [(t*6+i)*64 + vb]
            bias_sb = pp.tile([128, 6], F32)
            bst = pp.tile([128, 12], F32)
            ht_sb = pp.tile([128, KT * VB], BF16)    # gathered hT [kt*64+vb]
            h_loc = pp.tile([128, 2 * VB], F32)      # own hT slice [j*64+vb]
            hbf = pp.tile([128, 2 * VB], BF16)

            w_view = w_d.rearrange("(kt p) g -> kt p g", p=128)
            u_view = u_d.rearrange("(kt p) g -> kt p g", p=128)
            for kt in range(KT):
                stu = sp.tile([128, GPC], F32, name="stu")
                nc.sync.dma_start(stu[:], u_view[kt])
                nc.any.tensor_copy(u_sb[:, kt * GPC:(kt + 1) * GPC], stu[:])
            for kt in range(KT):
                stw = sp.tile([128, GPC], F32, name="stw")
                nc.sync.dma_start(stw[:], w_view[kt])
                nc.any.tensor_copy(w_sb[:, kt * GPC:(kt + 1) * GPC], stw[:])

            stx = sp.tile([128, KT * 256], F32, name="stx")
            nc.sync.dma_start(
                stx[:].rearrange("p (kk c) -> p kk c", kk=KT), xt_d)
            nc.any.tensor_copy(xt_sb[:], stx[:])

            # bias: bias_sb = b_in + b_rec (z/r gates); bst kept for h gate
            nc.sync.dma_start(bst[:], b_d)
            nc.vector.tensor_add(bias_sb[:], bst[:, 0:6], bst[:, 6:12])

            # h0 -> h_loc and ht_sb (broadcast over the 8 mask variants)
            h0s = sp.tile([128, KT * B], F32, name="h0s")
            nc.sync.dma_start(
                h0s[:].rearrange("p (kt b) -> p kt b", kt=KT), h0t_d)
            ht4 = ht_sb[:].rearrange("p (kt vb) -> p kt vb", kt=KT)
            for v in range(M):
                nc.any.tensor_copy(
                    ht4[:, :, v * B:(v + 1) * B],
                    h0s[:].rearrange("p (kt b) -> p kt b", kt=KT))
            h0ks = sp.tile([128, 2 * B], F32, name="h0ks")
            nc.sync.dma_start(
                h0ks[:].rearrange("p (j b) -> p j b", j=2), h0k_d)
            hl3 = h_loc[:].rearrange("p (j vb) -> p j vb", j=2)
            for v in range(M):
                nc.any.tensor_copy(
                    hl3[:, :, v * B:(v + 1) * B],
                    h0ks[:].rearrange("p (j b) -> p j b", j=2))

            # ---- input projection with mask-cumsum ----
            with tc.tile_pool(name="accp", bufs=1, space="PSUM") as accp:
                accs = [accp.tile([128, 256], F32, name=f"acc{mt}")
                        for mt in range(6)]
                xp4 = xp_sb[:].rearrange("p (t i vb) -> p t i vb", t=T, i=6)
                for j in range(M):
                    for kd in range(2):
                        kk = j * 2 + kd
                        for mt in range(6):
                            nc.tensor.matmul(
                                accs[mt][:],
                                w_sb[:, kk * GPC + mt * 128:
                                     kk * GPC + (mt + 1) * 128],
                                xt_sb[:, kk * 256:(kk + 1) * 256],
                                start=(kk == 0), stop=(kk == 2 * M - 1))
                    for mt in range(6):
                        nc.scalar.activation(
                            xp4[:, :, mt, j * B:(j + 1) * B],
                            accs[mt][:].rearrange("p (b t) -> p t b", b=B),
                            AF.Copy)

            # ---- recurrent steps ----
            with tc.tile_pool(name="psum", bufs=2, space="PSUM") as psp:
                for t in range(T):
                    if t != 0:  # t==0 uses the preloaded h0 gather
                        agi = dp.tile([2 * 128, VB], BF16, name=f"agi_{t}")
                        ago = dp.tile([KT * 128, VB], BF16, name=f"ago_{t}",
                                      addr_space="Shared")
                        nc.sync.dma_start(
                            agi[:].rearrange("(j p) vb -> p j vb", p=128),
                            hbf[:].rearrange("p (j vb) -> p j vb", j=2))
                        nc.gpsimd.collective_compute(
                            "AllGather", mybir.AluOpType.bypass,
                            replica_groups=[list(range(NCORES))],
                            ins=[agi.opt()], outs=[ago.opt()])
                        nc.sync.dma_start(
                            ht_sb[:].rearrange("p (kt vb) -> p kt vb", kt=KT),
                            ago[:].rearrange("(kt p) vb -> p kt vb", p=128))
                    recA = psp.tile([128, 4 * VB], F32, name="recA")
                    recB = psp.tile([128, 2 * VB], F32, name="recB")
                    for mt in range(6):
                        out = (recA[:, mt * VB:(mt + 1) * VB] if mt < 4
                               else recB[:, (mt - 4) * VB:(mt - 3) * VB])
                        for kt in range(KT):
                            nc.tensor.matmul(
                                out,
                                u_sb[:, kt * GPC + mt * 128:
                                     kt * GPC + (mt + 1) * 128],
                                ht_sb[:, kt * VB:(kt + 1) * VB],
                                start=(kt == 0), stop=(kt == KT - 1))
                    xpt = xp_sb[:, t * 6 * VB:(t + 1) * 6 * VB]
                    tmp_zr = wp.tile([128, 4 * VB], F32, name="tmp_zr")
                    nc.vector.tensor_add(tmp_zr[:], recA[:], xpt[:, 0:4 * VB])
                    zr = wp.tile([128, 4 * VB], F32, name="zr")
                    for i in range(4):
                        nc.scalar.activation(
                            zr[:, i * VB:(i + 1) * VB],
                            tmp_zr[:, i * VB:(i + 1) * VB],
                            AF.Sigmoid, bias=bias_sb[:, i:i + 1])
                    tmp_g = wp.tile([128, 2 * VB], F32, name="tmp_g")
                    for i in range(2):  # rh + b_rec_h (before r-multiply)
                        nc.vector.tensor_scalar_add(
                            tmp_g[:, i * VB:(i + 1) * VB],
                            recB[:, i * VB:(i + 1) * VB],
                            bst[:, 10 + i:11 + i])
                    nc.vector.tensor_mul(tmp_g[:], zr[:, 2 * VB:4 * VB],
                                         tmp_g[:])
                    nc.vector.tensor_add(tmp_g[:], tmp_g[:],
                                         xpt[:, 4 * VB:6 * VB])
                    hh = wp.tile([128, 2 * VB], F32, name="hh")
                    for i in range(2):  # tanh(. + b_in_h)
                        nc.scalar.activation(
                            hh[:, i * VB:(i + 1) * VB],
                            tmp_g[:, i * VB:(i + 1) * VB],
                            AF.Tanh, bias=bst[:, 4 + i:5 + i])
                    dmh = wp.tile([128, 2 * VB], F32, name="dmh")
                    nc.vector.tensor_sub(dmh[:], h_loc[:], hh[:])
                    nc.vector.tensor_mul(dmh[:], zr[:, 0:2 * VB], dmh[:])
                    nc.vector.tensor_add(h_loc[:], dmh[:], hh[:])
                    nc.vector.tensor_copy(hbf[:], h_loc[:])
                    nc.sync.dma_start(
                        hs_d[t].rearrange("j p vb -> p j vb"),
                        h_loc[:].rearrange("p (j vb) -> p j vb", j=2))
    nc.compile()
    return nc


def _make_in_maps(x, h0, W, U, b):
    # xt[p, j*2+kd, b*32+t] = x[b,t,j,kd*128+p]
    xt = np.ascontiguousarray(
        x.reshape(B, T, M, 2, 128).transpose(4, 2, 3, 0, 1)
        .reshape(128, KT, 256))
    # h0t[p, kt, b] = h0[b, kt*128+p]
    h0t = np.ascontiguousarray(h0.T.reshape(KT, 128, B).transpose(1, 0, 2))
    in_maps = []
    for k in range(NCORES):
        cols = np.concatenate([np.arange(g * UNITS + k * 256,
                                         g * UNITS + (k + 1) * 256)
                               for g in range(3)])
        h0k = np.ascontiguousarray(
            h0[:, k * 256:(k + 1) * 256].T.reshape(2, 128, B)
            .transpose(1, 0, 2))
        bk = np.ascontiguousarray(
            b[:, cols].reshape(2, 3, 2, 128).transpose(3, 0, 1, 2)
            .reshape(128, 12))
        in_maps.append({
            "xt": xt,
            "h0t": h0t,
            "h0k": h0k,
            "wk": np.ascontiguousarray(W[:, cols]),
            "uk": np.ascontiguousarray(U[:, cols]),
            "bk": bk,
        })
    return in_maps


def kernel(x, h0, W, U, b):
    x = np.asarray(x, np.float32)
    h0 = np.asarray(h0, np.float32)
    W = np.asarray(W, np.float32)
    U = np.asarray(U, np.float32)
    b = np.asarray(b, np.float32)

    from concourse import bass_utils
    if "nc" not in _CACHE:
        _CACHE["nc"] = _build()
    nc = _CACHE["nc"]
    in_maps = _make_in_maps(x, h0, W, U, b)
    res = bass_utils.run_bass_kernel_spmd(
        nc, in_maps, core_ids=list(range(NCORES)))

    ret = np.empty((B, T, M, D), np.float32)
    hs = []
    for k in range(NCORES):
        a = np.asarray(res.results[k]["hs"], np.float32)
        a = a.transpose(0, 3, 1, 2).reshape(T, VB, D)  # [t, vb, d]
        hs.append(a)
        # ret[b,t,k,:] = h trajectory of variant k, feature slice k
        ret[:, :, k, :] = a[:, k * B:(k + 1) * B, :].transpose(1, 0, 2)
    state = np.concatenate(
        [hs[k][T - 1, (M - 1) * B:M * B, :] for k in range(NCORES)], axis=1)
    return ret, state


# revision 2
# speedup vs baseline: 1.0696x; 1.0696x over previous
"""Masked multi-variant GRU (nn_MiddleLayer_30545807409998) on 8 TRN2 cores.

Strategy (self-contained; shapes hardcoded):
- Tensor-parallel over the 3*units gate dimension: core k owns gate columns
  [k*256,(k+1)*256) of each of z/r/h (768 U/W columns, host pre-sliced) and
  the matching h slice (rows [k*256,(k+1)*256) of h^T).
- Per time step: AllGather of the bf16 h^T slices across the 8 cores, then
  rec^T = (U_k)^T @ h^T on the TensorEngine (stationary = U tiles, full
  128x128), gate math on ACT/DVE in feature-major layout
  [feature partitions, vb free], vb = variant*8 + batch (64 columns).
- Input projection uses the mask-cumsum identity: the M=8 lower-triangular
  mask variants satisfy xp_v = sum_{j<=v} x_j @ W_j (j = maxlen block), an
  8x FLOP saving; partial sums accumulate in PSUM and each prefix is
  snapshotted to SBUF.
- Keras reset_after GRU bias handling: z/r gates get (b_in+b_rec) inside the
  sigmoid; the h gate adds b_rec BEFORE the r* multiply and b_in inside tanh.
- Matmuls in bf16 (fp32 PSUM accumulation), gate math and h state in fp32;
  measured absmax relative error ~7e-3 vs the fp32 reference.

Host side (this file): slices weight columns per core, transposes x/h0 to
the device layouts (pure gathers), launches the SPMD program via
run_bass_kernel_spmd, and reassembles ret/state from per-core outputs.
"""
import numpy as np

NCORES = 8
B, T, M, D = 8, 32, 8, 256
UNITS = M * D              # 2048
GPC = 3 * UNITS // NCORES  # 768 gate cols per core
VB = M * B                 # 64 folded (variant, batch)
KT = UNITS // 128          # 16 k-tiles

_CACHE = {}


def _build(repeat=1):
    import concourse.bacc as bacc
    import concourse.tile as tile
    import concourse.mybir as mybir
    F32 = mybir.dt.float32
    BF16 = mybir.dt.bfloat16
    AF = mybir.ActivationFunctionType

    nc = bacc.Bacc("TRN2", target_bir_lowering=False, debug=False,
                   num_devices=NCORES)
    xt_d = nc.dram_tensor("xt", [128, KT, 256], F32, kind="ExternalInput").ap()
    h0t_d = nc.dram_tensor("h0t", [128, KT, B], F32,
                           kind="ExternalInput").ap()
    h0k_d = nc.dram_tensor("h0k", [128, 2, B], F32, kind="ExternalInput").ap()
    w_d = nc.dram_tensor("wk", [UNITS, GPC], F32, kind="ExternalInput").ap()
    u_d = nc.dram_tensor("uk", [UNITS, GPC], F32, kind="ExternalInput").ap()
    b_d = nc.dram_tensor("bk", [128, 12], F32, kind="ExternalInput").ap()
    hs_d = nc.dram_tensor("hs", [T, 128, 2 * VB], F32,
                          kind="ExternalOutput").ap()

    with tile.TileContext(nc) as tc:
        with tc.tile_pool(name="persist", bufs=1) as pp, \
             tc.tile_pool(name="stage", bufs=2) as sp, \
             tc.tile_pool(name="work", bufs=3) as wp, \
             tc.tile_pool(name="dram", bufs=1, space="DRAM") as dp:

            u_sb = pp.tile([128, KT * GPC], BF16)
            w_sb = pp.tile([128, KT * GPC], BF16)
            xt_sb = pp.tile([128, KT * 256], BF16)   # [(j,kd)*256 + b*32+t]
            xp_sb = pp.tile([128, T * 6 * VB], F32)  # [(t*6+i)*64 + vb]
            bias_sb = pp.tile([128, 6], F32)
            bst = pp.tile([128, 12], F32)
            ht_sb = pp.tile([128, KT * VB], BF16)    # gathered hT [kt*64+vb]
            h_loc = pp.tile([128, 2 * VB], F32)      # own hT slice [j*64+vb]
            hbf = pp.tile([128, 2 * VB], BF16)

            w_view = w_d.rearrange("(kt p) g -> kt p g", p=128)
            u_view = u_d.rearrange("(kt p) g -> kt p g", p=128)
            for kt in range(KT):
                stu = sp.tile([128, GPC], F32, name="stu")
                nc.sync.dma_start(stu[:], u_view[kt])
                nc.any.tensor_copy(u_sb[:, kt * GPC:(kt + 1) * GPC], stu[:])
            for kt in range(KT):
                stw = sp.tile([128, GPC], F32, name="stw")
                nc.sync.dma_start(stw[:], w_view[kt])
                nc.any.tensor_copy(w_sb[:, kt * GPC:(kt + 1) * GPC], stw[:])

            stx = sp.tile([128, KT * 256], F32, name="stx")
            nc.sync.dma_start(
                stx[:].rearrange("p (kk c) -> p kk c", kk=KT), xt_d)
            nc.any.tensor_copy(xt_sb[:], stx[:])

            # bias: bias_sb = b_in + b_rec (z/r gates); bst kept for h gate
            nc.sync.dma_start(bst[:], b_d)
            nc.vector.tensor_add(bias_sb[:], bst[:, 0:6], bst[:, 6:12])

            # h0 -> h_loc and ht_sb (broadcast over the 8 mask variants)
            h0s = sp.tile([128, KT * B], F32, name="h0s")
            nc.sync.dma_start(
                h0s[:].rearrange("p (kt b) -> p kt b", kt=KT), h0t_d)
            ht4 = ht_sb[:].rearrange("p (kt vb) -> p kt vb", kt=KT)
            for v in range(M):
                nc.any.tensor_copy(
                    ht4[:, :, v * B:(v + 1) * B],
                    h0s[:].rearrange("p (kt b) -> p kt b", kt=KT))
            h0ks = sp.tile([128, 2 * B], F32, name="h0ks")
            nc.sync.dma_start(
                h0ks[:].rearrange("p (j b) -> p j b", j=2), h0k_d)
            hl3 = h_loc[:].rearrange("p (j vb) -> p j vb", j=2)
            for v in range(M):
                nc.any.tensor_copy(
                    hl3[:, :, v * B:(v + 1) * B],
                    h0ks[:].rearrange("p (j b) -> p j b", j=2))

            # ---- input projection with mask-cumsum ----
            with tc.tile_pool(name="accp", bufs=1, space="PSUM") as accp:
                accs = [accp.tile([128, 256], F32, name=f"acc{mt}")
                        for mt in range(6)]
                xp4 = xp_sb[:].rearrange("p (t i vb) -> p t i vb", t=T, i=6)
                for j in range(M):
                    for kd in range(2):
                        kk = j * 2 + kd
                        for mt in range(6):
                            nc.tensor.matmul(
                                accs[mt][:],
                                w_sb[:, kk * GPC + mt * 128:
                                     kk * GPC + (mt + 1) * 128],
                                xt_sb[:, kk * 256:(kk + 1) * 256],
                                start=(kk == 0), stop=(kk == 2 * M - 1))
                    for mt in range(6):
                        nc.scalar.activation(
                            xp4[:, :, mt, j * B:(j + 1) * B],
                            accs[mt][:].rearrange("p (b t) -> p t b", b=B),
                            AF.Copy)

            # ---- recurrent steps ----
            with tc.tile_pool(name="psum", bufs=2, space="PSUM") as psp:
              for r in range(repeat):
                for t in range(T):
                    if not (r == 0 and t == 0):  # t==0 uses preloaded h0
                        # partition-major payload: agi[p, j*64+vb]; rank
                        # block k of the gather then lands exactly on
                        # ht_sb[:, k*128:(k+1)*128] -- all DMAs are 2D
                        # contiguous (no fine-grained scatter descriptors)
                        agi = dp.tile([128, 2 * VB], BF16, name=f"agi_{r}_{t}")
                        ago = dp.tile([NCORES * 128, 2 * VB], BF16,
                                      name=f"ago_{r}_{t}", addr_space="Shared")
                        nc.sync.dma_start(agi[:], hbf[:])
                        nc.gpsimd.collective_compute(
                            "AllGather", mybir.AluOpType.bypass,
                            replica_groups=[list(range(NCORES))],
                            ins=[agi.opt()], outs=[ago.opt()])
                        nc.sync.dma_start(
                            ht_sb[:].rearrange("p (k c) -> p k c", k=NCORES),
                            ago[:].rearrange("(k p) c -> p k c", p=128))
                    recA = psp.tile([128, 4 * VB], F32, name="recA")
                    recB = psp.tile([128, 2 * VB], F32, name="recB")
                    for mt in range(6):
                        out = (recA[:, mt * VB:(mt + 1) * VB] if mt < 4
                               else recB[:, (mt - 4) * VB:(mt - 3) * VB])
                        for kt in range(KT):
                            nc.tensor.matmul(
                                out,
                                u_sb[:, kt * GPC + mt * 128:
                                     kt * GPC + (mt + 1) * 128],
                                ht_sb[:, kt * VB:(kt + 1) * VB],
                                start=(kt == 0), stop=(kt == KT - 1))
                    xpt = xp_sb[:, t * 6 * VB:(t + 1) * 6 * VB]
                    tmp_zr = wp.tile([128, 4 * VB], F32, name="tmp_zr")
                    nc.vector.tensor_add(tmp_zr[:], recA[:], xpt[:, 0:4 * VB])
                    zr = wp.tile([128, 4 * VB], F32, name="zr")
                    for i in range(4):
                        nc.scalar.activation(
                            zr[:, i * VB:(i + 1) * VB],
                            tmp_zr[:, i * VB:(i + 1) * VB],
                            AF.Sigmoid, bias=bias_sb[:, i:i + 1])
                    tmp_g = wp.tile([128, 2 * VB], F32, name="tmp_g")
                    for i in range(2):  # rh + b_rec_h (before r-multiply)
                        nc.vector.tensor_scalar_add(
                            tmp_g[:, i * VB:(i + 1) * VB],
                            recB[:, i * VB:(i + 1) * VB],
                            bst[:, 10 + i:11 + i])
                    nc.vector.tensor_mul(tmp_g[:], zr[:, 2 * VB:4 * VB],
                                         tmp_g[:])
                    nc.vector.tensor_add(tmp_g[:], tmp_g[:],
                                         xpt[:, 4 * VB:6 * VB])
                    hh = wp.tile([128, 2 * VB], F32, name="hh")
                    for i in range(2):  # tanh(. + b_in_h)
                        nc.scalar.activation(
                            hh[:, i * VB:(i + 1) * VB],
                            tmp_g[:, i * VB:(i + 1) * VB],
                            AF.Tanh, bias=bst[:, 4 + i:5 + i])
                    dmh = wp.tile([128, 2 * VB], F32, name="dmh")
                    nc.vector.tensor_sub(dmh[:], h_loc[:], hh[:])
                    nc.vector.tensor_mul(dmh[:], zr[:, 0:2 * VB], dmh[:])
                    nc.vector.tensor_add(h_loc[:], dmh[:], hh[:])
                    nc.vector.tensor_copy(hbf[:], h_loc[:])
                    nc.sync.dma_start(hs_d[t], h_loc[:])
    nc.compile()
    return nc


def _make_in_maps(x, h0, W, U, b):
    # xt[p, j*2+kd, b*32+t] = x[b,t,j,kd*128+p]
    xt = np.ascontiguousarray(
        x.reshape(B, T, M, 2, 128).transpose(4, 2, 3, 0, 1)
        .reshape(128, KT, 256))
    # h0t[p, kt, b] = h0[b, kt*128+p]
    h0t = np.ascontiguousarray(h0.T.reshape(KT, 128, B).transpose(1, 0, 2))
    in_maps = []
    for k in range(NCORES):
        cols = np.concatenate([np.arange(g * UNITS + k * 256,
                                         g * UNITS + (k + 1) * 256)
                               for g in range(3)])
        h0k = np.ascontiguousarray(
            h0[:, k * 256:(k + 1) * 256].T.reshape(2, 128, B)
            .transpose(1, 0, 2))
        bk = np.ascontiguousarray(
            b[:, cols].reshape(2, 3, 2, 128).transpose(3, 0, 1, 2)
            .reshape(128, 12))
        in_maps.append({
            "xt": xt,
            "h0t": h0t,
            "h0k": h0k,
            "wk": np.ascontiguousarray(W[:, cols]),
            "uk": np.ascontiguousarray(U[:, cols]),
            "bk": bk,
        })
    return in_maps


def kernel(x, h0, W, U, b):
    x = np.asarray(x, np.float32)
    h0 = np.asarray(h0, np.float32)
    W = np.asarray(W, np.float32)
    U = np.asarray(U, np.float32)
    b = np.asarray(b, np.float32)

    from concourse import bass_utils
    if "nc" not in _CACHE:
        _CACHE["nc"] = _build()
    nc = _CACHE["nc"]
    in_maps = _make_in_maps(x, h0, W, U, b)
    res = bass_utils.run_bass_kernel_spmd(
        nc, in_maps, core_ids=list(range(NCORES)))

    ret = np.empty((B, T, M, D), np.float32)
    hs = []
    for k in range(NCORES):
        a = np.asarray(res.results[k]["hs"], np.float32)
        # [t, p, j*64+vb] -> [t, vb, j*128+p]
        a = a.reshape(T, 128, 2, VB).transpose(0, 3, 2, 1).reshape(T, VB, D)
        hs.append(a)
        # ret[b,t,k,:] = h trajectory of variant k, feature slice k
        ret[:, :, k, :] = a[:, k * B:(k + 1) * B, :].transpose(1, 0, 2)
    state = np.concatenate(
        [hs[k][T - 1, (M - 1) * B:M * B, :] for k in range(NCORES)], axis=1)
    return ret, state
